# revision 13
# baseline (speedup 1.0000x reference)
"""Trainium2 Bass kernel for nn_Long_LSTM_Top (2-window masked LSTM + sum-pool + FC).

Strategy (hardcoded for B=256, T=300, C=128, H=256, CLS=60, windows at p=0 and
p=145, each 154 long, over the lag-1 difference sequence d[p] = x[p+1]-x[p]):

- Data-parallel over batch across 8 cores (32 batch rows/core).
- Both windows' LSTM chains run fused: feature dim on partitions and
  (row, window) = 64 columns in the free dim; each weight tile is loaded
  once per step for both windows.
- Weights arrive pre-transposed fp16 from the host; x arrives fp16 and is
  transposed to [c, (r t)] by a single XBAR DMA-transpose. Prep is 4 DMAs
  plus four wide lag-difference subs.
- Per step the 8 gate chunks live in one 2-bank psum tile (half x at block
  offset 8x). The 8 input-projection matmuls lead the banks (first per bank
  start=True, rest overwrite via cleared has_written) and hoist out of the
  h-critical path; W_hh partials run k0-first so the k0 burst of the next
  step overlaps the tail of half-1's chain.
- tanh(g) is folded into the matmul (g rows doubled on host): one merged
  sigmoid over both banks covers all 8 gate blocks; tanh(g) = 2*sg-1,
  c update via paired multiply + pair add, all merged across halves.
  tanh(c) and h = so*tanh(c) stay split per half (h0 lands one ACT+mul
  earlier than h1, feeding the k0 burst early).
- Matmul operands fp16, c state fp32, h fp16. Final FC in fp32.
"""

import numpy as np

import concourse.bass as bass
import concourse.mybir as mybir
from concourse import bacc
from concourse.tile import TileContext

F32 = mybir.dt.float32
F16 = mybir.dt.float16

B, T, C, H, CLS = 256, 300, 128, 256, 60
START, STRIDE, WIN = 1, 145, 154
NUM_WIN = 2
L = T - START  # 299
NCORES = 8
BC = B // NCORES  # 32 rows per core
NSTEP = L  # 299 wall steps
NCOL = NUM_WIN * BC  # 64 scan columns, (row, window) order

# Bank block order [g, f, i, o]; PyTorch gate chunk index for (gate, half x):
# i -> 0+x, f -> 2+x, g -> 4+x, o -> 6+x.
BLK_GATE = [4, 2, 0, 6]


def build(bias_zero: bool = True, nstep: int = NSTEP):
    """Build the per-core Bass module. Returns nc."""
    nc = bacc.Bacc("TRN2", target_bir_lowering=False, debug=False)

    x_d = nc.declare_dram_parameter("x", [BC * T, C], F16, isOutput=False)
    # host-pre-transposed weights: wihT col block = gate chunk,
    # whhT col block = g*2+k, wfcT col block = feature chunk.
    wihT_d = nc.declare_dram_parameter("wihT", [128, 8 * 128], F16, isOutput=False)
    whhT_d = nc.declare_dram_parameter("whhT", [128, 16 * 128], F16, isOutput=False)
    wfcT_d = nc.declare_dram_parameter("wfcT", [128, 4 * CLS], F32, isOutput=False)
    bias_d = nc.declare_dram_parameter("bias", [4 * H], F32, isOutput=False)
    out_d = nc.declare_dram_parameter("out", [CLS, BC], F32, isOutput=True)

    with TileContext(nc) as tc:
        with (
            tc.tile_pool(name="persist", bufs=1) as persist,
            tc.tile_pool(name="ps", bufs=3, space="PSUM") as ps_pool,
            tc.tile_pool(name="fc_ps", bufs=1, space="PSUM") as fc_ps,
            tc.tile_pool(name="sigp", bufs=3) as sigp,
            tc.tile_pool(name="ctgp", bufs=3) as ctgp,
            tc.tile_pool(name="prodp", bufs=3) as prodp,
            tc.tile_pool(name="tcp", bufs=3) as tcp,
            tc.tile_pool(name="hp", bufs=3) as hp,
        ):
            whhT = persist.tile([128, 16 * 128], F16)
            nc.sync.dma_start(out=whhT, in_=whhT_d[:])
            wihT = persist.tile([128, 8 * 128], F16)
            nc.sync.dma_start(out=wihT, in_=wihT_d[:])
            wfcT = persist.tile([128, 4 * CLS], F32)
            nc.sync.dma_start(out=wfcT, in_=wfcT_d[:])

            # x: single XBAR DMA-transpose [BC*T, C] -> [c, (r t)]
            xT = persist.tile([128, BC * T], F16)
            nc.sync.dma_start_transpose(xT, x_d[:])

            bias_sb = None
            if not bias_zero:
                bias_sb = persist.tile([128, 8], F32)
                nc.sync.dma_start(
                    out=bias_sb, in_=bias_d[:].rearrange("(g p) -> p g", p=128)
                )

            # ---- masked lag-difference, fp16, layout [c, r, win, t] ------
            dm = persist.tile([128, BC, NUM_WIN, NSTEP], F16)
            nc.gpsimd.memset(dm, 0.0)
            xT3 = xT[:].rearrange("p (r t) -> p r t", r=BC)
            # window 0 active at p in [0, 154); window 1 at p in [145, 299).
            for (wwin, lo, hi) in [(0, 0, 80), (1, STRIDE, 224), (0, 80, WIN),
                                   (1, 224, L)]:
                nc.vector.tensor_sub(
                    dm[:, :, wwin, lo:hi],
                    xT3[:, :, lo + 1:hi + 1],
                    xT3[:, :, lo:hi],
                )

            # ---- scan ----------------------------------------------------
            pooled = persist.tile([128, 2, NCOL], F32)
            nc.gpsimd.memset(pooled, 0.0)
            h_prev = hp.tile([128, 2, NCOL], F16, tag="h")
            nc.gpsimd.memset(h_prev, 0.0)
            # ctg tile: per half x, [c_prev, tanh_g] pair blocks.
            ctg_cur = ctgp.tile([128, 2, 2, NCOL], F32, tag="ctg")
            nc.gpsimd.memset(ctg_cur[:, :, 0, :], 0.0)

            sig = mybir.ActivationFunctionType.Sigmoid
            tnh = mybir.ActivationFunctionType.Tanh

            for w in range(nstep):
                rhs_d = dm[:, :, :, w]
                ctg_next = ctgp.tile([128, 2, 2, NCOL], F32, tag="ctg")
                sg = sigp.tile([128, 2, 4, NCOL], F32, tag="sg")
                prod = prodp.tile([128, 2, 2, NCOL], F32, tag="pr")
                tcn = tcp.tile([128, 2, NCOL], F32, tag="tc")
                hn = hp.tile([128, 2, NCOL], F16, tag="h")
                # One 2-bank tile: half x at block offset 8x, blocks [g,f,i,o].
                ps = ps_pool.tile([128, 16, NCOL], F32, tag="ps")
                # Input-projection matmuls lead each bank: the first clears
                # the bank (start=True), the rest overwrite fresh regions
                # (has_written clear), so all 8 run before h(t-1) exists.
                for x in (0, 1):
                    for b in range(4):
                        gc = BLK_GATE[b] + x
                        nc.tensor.matmul(
                            out=ps[:, 8 * x + b, :],
                            lhsT=wihT[:, gc * 128:(gc + 1) * 128],
                            rhs=rhs_d,
                            start=(b == 0),
                            stop=False,
                            skip_group_check=True,
                        )
                # W_hh partials: all k0 (h half 0 lands one stage before
                # half 1) then all k1.
                for k in (0, 1):
                    for x in (0, 1):
                        for b in range(4):
                            gc = BLK_GATE[b] + x
                            nc.tensor.matmul(
                                out=ps[:, 8 * x + b, :],
                                lhsT=whhT[:, (gc * 2 + k) * 128:(gc * 2 + k + 1) * 128],
                                rhs=h_prev[:, k, :],
                                start=False,
                                stop=(k == 1),
                                skip_group_check=True,
                            )

                # Merged sigmoid over both banks covers all 8 gate blocks
                # (g rows doubled host-side: block 0 holds sigma(2g)).
                ps4 = ps[:].rearrange("p (v b) c -> p v b c", v=2)
                if bias_zero:
                    nc.scalar.activation(sg[:], ps4[:, :, 0:4, :], sig)
                else:
                    for x in (0, 1):
                        for b in range(4):
                            nc.scalar.activation(
                                sg[:, x, b, :], ps[:, 8 * x + b, :], sig,
                                bias=bias_sb[:, BLK_GATE[b] + x:BLK_GATE[b] + x + 1],
                            )
                # tanh(g) = 2*sigma(2g) - 1, merged across halves
                nc.vector.tensor_scalar(
                    ctg_cur[:, :, 1, :], sg[:, :, 0, :], 2.0, -1.0,
                    mybir.AluOpType.mult, mybir.AluOpType.add,
                )
                # prod = [sf*c_prev, si*tanh_g], both halves in one op
                nc.vector.tensor_mul(prod[:], sg[:, :, 1:3, :], ctg_cur[:, :, :, :])
                nc.vector.tensor_add(
                    ctg_next[:, :, 0, :], prod[:, :, 0, :], prod[:, :, 1, :]
                )
                # tanh(c) and h split per half: h0 lands early, feeds k0 burst
                nc.scalar.activation(tcn[:, 0, :], ctg_next[:, 0, 0, :], tnh)
                nc.gpsimd.tensor_mul(hn[:, 0, :], sg[:, 0, 3, :], tcn[:, 0, :])
                nc.scalar.activation(tcn[:, 1, :], ctg_next[:, 1, 0, :], tnh)
                nc.vector.tensor_mul(hn[:, 1, :], sg[:, 1, 3, :], tcn[:, 1, :])

                nc.gpsimd.tensor_add(pooled, pooled, hn)
                h_prev = hn
                ctg_cur = ctg_next

            # ---- FC ------------------------------------------------------
            # pooled columns are (row, window)-ordered; FC consumes
            # window-major slices via a strided AP (one-shot, cost ok).
            pooled4 = pooled[:].rearrange("p k (r v) -> p k r v", v=NUM_WIN)
            fps = fc_ps.tile([CLS, BC], F32, tag="fc")
            for idx, (cw, k) in enumerate([(0, 0), (0, 1), (1, 0), (1, 1)]):
                nc.tensor.matmul(
                    out=fps,
                    lhsT=wfcT[:, idx * CLS:(idx + 1) * CLS],
                    rhs=pooled4[:, k, :, cw],
                    start=(idx == 0),
                    stop=(idx == 3),
                )
            out_sb = persist.tile([CLS, BC], F32)
            nc.scalar.copy(out=out_sb, in_=fps)
            nc.sync.dma_start(out=out_d[:], in_=out_sb)

    nc.finalize()
    return nc


_CACHE = {}


def _get_nc(bias_zero: bool):
    if bias_zero not in _CACHE:
        _CACHE[bias_zero] = build(bias_zero)
    return _CACHE[bias_zero]


def host_weights(W_ih, W_hh, W_fc, bias):
    """Host-side weight prep: fold tanh(g)=2*sigmoid(2g)-1 (double g rows),
    transpose into the kernel's tile layouts, cast matmul operands fp16."""
    W_ih = np.asarray(W_ih, np.float32).copy()
    W_hh = np.asarray(W_hh, np.float32).copy()
    bias = np.asarray(bias, np.float32).copy()
    W_ih[2 * H:3 * H] *= 2.0
    W_hh[2 * H:3 * H] *= 2.0
    bias[2 * H:3 * H] *= 2.0
    wihT = np.ascontiguousarray(W_ih.T).astype(np.float16)  # [128, 8*128]
    # whhT[p, (g*2+k)*128 + m] = W_hh[g*128 + m, k*128 + p]
    w = W_hh.reshape(8, 128, 2, 128)  # [g, m, k, p]
    whhT = np.ascontiguousarray(
        w.transpose(3, 0, 2, 1).reshape(128, 16 * 128)
    ).astype(np.float16)
    # wfcT[p, idx*CLS + cls] = W_fc[cls, idx*128 + p]
    wfcT = np.ascontiguousarray(
        np.asarray(W_fc, np.float32).T.reshape(4, 128, CLS)
        .transpose(1, 0, 2).reshape(128, 4 * CLS)
    )
    return wihT, whhT, wfcT, bias


def kernel(x, W_ih, W_hh, b_ih, b_hh, W_fc, b_fc):
    from concourse.bass_utils import run_bass_kernel_spmd

    x = np.asarray(x, dtype=np.float32)
    b_fc = np.asarray(b_fc, dtype=np.float32)
    bias = np.asarray(b_ih, np.float32) + np.asarray(b_hh, np.float32)
    bias_zero = bool(np.all(bias == 0.0))
    nc = _get_nc(bias_zero)

    wihT, whhT, wfcT, bias = host_weights(W_ih, W_hh, W_fc, bias)
    x16 = x.astype(np.float16)

    in_maps = []
    for c in range(NCORES):
        xc = np.ascontiguousarray(x16[c * BC:(c + 1) * BC].reshape(BC * T, C))
        in_maps.append(
            {"x": xc, "wihT": wihT, "whhT": whhT, "wfcT": wfcT, "bias": bias}
        )

    res = run_bass_kernel_spmd(nc, in_maps, list(range(NCORES)))
    out = np.concatenate([r["out"].T for r in res.results], axis=0)
    return (out + b_fc[None, :]).astype(np.float32)


# revision 14
# speedup vs baseline: 1.1652x; 1.1652x over previous
"""Trainium2 Bass kernel for nn_Long_LSTM_Top (2-window masked LSTM + sum-pool + FC).

Strategy (hardcoded for B=256, T=300, C=128, H=256, CLS=60, windows at p=0 and
p=145, each 154 long, over the lag-1 difference sequence d[p] = x[p+1]-x[p]):

- Data-parallel over batch across 8 cores (32 batch rows/core).
- Both windows' LSTM chains run fused: feature dim on partitions and
  (row, window) = 64 columns in the free dim; each weight tile is loaded
  once per step for both windows.
- Weights arrive pre-transposed fp16 from the host; x arrives fp16 and is
  transposed to [c, (r t)] by a single XBAR DMA-transpose. Prep is 4 DMAs
  plus four wide lag-difference subs.
- Per step the 8 gate chunks live in one 2-bank psum tile (half x at block
  offset 8x). The 8 input-projection matmuls lead the banks (first per bank
  start=True, rest overwrite via cleared has_written) and hoist out of the
  h-critical path; W_hh partials run k0-first so the k0 burst of the next
  step overlaps the tail of half-1's chain.
- tanh(g) is folded into the matmul (g rows doubled on host): one merged
  sigmoid over both banks covers all 8 gate blocks; tanh(g) = 2*sg-1,
  c update via paired multiply + pair add, all merged across halves.
  tanh(c) and h = so*tanh(c) stay split per half (h0 lands one ACT+mul
  earlier than h1, feeding the k0 burst early).
- Matmul operands fp16, c state fp32, h fp16. Final FC in fp32.
"""

import numpy as np

import concourse.bass as bass
import concourse.mybir as mybir
from concourse import bacc
from concourse.tile import TileContext

F32 = mybir.dt.float32
F16 = mybir.dt.float16

B, T, C, H, CLS = 256, 300, 128, 256, 60
START, STRIDE, WIN = 1, 145, 154
NUM_WIN = 2
L = T - START  # 299
NCORES = 8
BC = B // NCORES  # 32 rows per core
NSTEP = L  # 299 wall steps
NCOL = NUM_WIN * BC  # 64 scan columns, (row, window) order

# Bank block order [g, f, i, o]; PyTorch gate chunk index for (gate, half x):
# i -> 0+x, f -> 2+x, g -> 4+x, o -> 6+x.
BLK_GATE = [4, 2, 0, 6]


def build(bias_zero: bool = True, nstep: int = NSTEP):
    """Build the per-core Bass module. Returns nc."""
    nc = bacc.Bacc("TRN2", target_bir_lowering=False, debug=False)

    x_d = nc.declare_dram_parameter("x", [BC * T, C], F16, isOutput=False)
    # host-pre-transposed weights: wihT col block = gate chunk,
    # whhT col block = g*2+k, wfcT col block = feature chunk.
    wihT_d = nc.declare_dram_parameter("wihT", [128, 8 * 128], F16, isOutput=False)
    whhT_d = nc.declare_dram_parameter("whhT", [128, 16 * 128], F16, isOutput=False)
    wfcT_d = nc.declare_dram_parameter("wfcT", [128, 4 * CLS], F32, isOutput=False)
    bias_d = nc.declare_dram_parameter("bias", [4 * H], F32, isOutput=False)
    out_d = nc.declare_dram_parameter("out", [CLS, BC], F32, isOutput=True)

    with TileContext(nc) as tc:
        with (
            tc.tile_pool(name="persist", bufs=1) as persist,
            tc.tile_pool(name="ps0", bufs=3, space="PSUM") as ps0_pool,
            tc.tile_pool(name="ps1", bufs=3, space="PSUM") as ps1_pool,
            tc.tile_pool(name="fc_ps", bufs=1, space="PSUM") as fc_ps,
            tc.tile_pool(name="sigp", bufs=3) as sigp,
            tc.tile_pool(name="ctgp", bufs=3) as ctgp,
            tc.tile_pool(name="prodp", bufs=3) as prodp,
            tc.tile_pool(name="tcp", bufs=3) as tcp,
            tc.tile_pool(name="hp", bufs=3) as hp,
        ):
            whhT = persist.tile([128, 16 * 128], F16)
            nc.sync.dma_start(out=whhT, in_=whhT_d[:])
            wihT = persist.tile([128, 8 * 128], F16)
            nc.sync.dma_start(out=wihT, in_=wihT_d[:])
            wfcT = persist.tile([128, 4 * CLS], F32)
            nc.sync.dma_start(out=wfcT, in_=wfcT_d[:])

            # x: single XBAR DMA-transpose [BC*T, C] -> [c, (r t)]
            xT = persist.tile([128, BC * T], F16)
            nc.sync.dma_start_transpose(xT, x_d[:])

            bias_sb = None
            if not bias_zero:
                bias_sb = persist.tile([128, 8], F32)
                nc.sync.dma_start(
                    out=bias_sb, in_=bias_d[:].rearrange("(g p) -> p g", p=128)
                )

            # ---- masked lag-difference, fp16, layout [c, r, win, t] ------
            dm = persist.tile([128, BC, NUM_WIN, NSTEP], F16)
            nc.gpsimd.memset(dm, 0.0)
            xT3 = xT[:].rearrange("p (r t) -> p r t", r=BC)
            # window 0 active at p in [0, 154); window 1 at p in [145, 299).
            for (wwin, lo, hi) in [(0, 0, 80), (1, STRIDE, 224), (0, 80, WIN),
                                   (1, 224, L)]:
                nc.vector.tensor_sub(
                    dm[:, :, wwin, lo:hi],
                    xT3[:, :, lo + 1:hi + 1],
                    xT3[:, :, lo:hi],
                )

            # ---- scan ----------------------------------------------------
            pooled = persist.tile([128, 2, NCOL], F32)
            nc.gpsimd.memset(pooled, 0.0)
            h_prev = hp.tile([128, 2, NCOL], F16, tag="h")
            nc.gpsimd.memset(h_prev, 0.0)
            # ctg tile: per half x, [c_prev, tanh_g] pair blocks.
            ctg_cur = ctgp.tile([128, 2, 2, NCOL], F32, tag="ctg")
            nc.gpsimd.memset(ctg_cur[:, :, 0, :], 0.0)

            sig = mybir.ActivationFunctionType.Sigmoid
            tnh = mybir.ActivationFunctionType.Tanh

            for w in range(nstep):
                rhs_d = dm[:, :, :, w]
                ctg_next = ctgp.tile([128, 2, 2, NCOL], F32, tag="ctg")
                sg = sigp.tile([128, 2, 4, NCOL], F32, tag="sg")
                prod = prodp.tile([128, 2, 2, NCOL], F32, tag="pr")
                tcn = tcp.tile([128, 2, NCOL], F32, tag="tc")
                hn = hp.tile([128, 2, NCOL], F16, tag="h")
                # Two banks: half x in its own bank, blocks [g,f,i,o].
                ps_t0 = ps0_pool.tile([128, 8, NCOL], F32, tag="ps")
                ps_t1 = ps1_pool.tile([128, 8, NCOL], F32, tag="ps")
                banks = (ps_t0, ps_t1)
                # Input-projection matmuls lead each bank: the first clears
                # the bank (start=True), the rest overwrite fresh regions
                # (has_written clear), so all 8 run before h(t-1) exists.
                for x in (0, 1):
                    for b in range(4):
                        gc = BLK_GATE[b] + x
                        nc.tensor.matmul(
                            out=banks[x][:, b, :],
                            lhsT=wihT[:, gc * 128:(gc + 1) * 128],
                            rhs=rhs_d,
                            start=(b == 0),
                            stop=False,
                            skip_group_check=True,
                        )
                # W_hh partials: all k0 (h half 0 lands one stage before
                # half 1) then all k1.
                for k in (0, 1):
                    for x in (0, 1):
                        for b in range(4):
                            gc = BLK_GATE[b] + x
                            nc.tensor.matmul(
                                out=banks[x][:, b, :],
                                lhsT=whhT[:, (gc * 2 + k) * 128:(gc * 2 + k + 1) * 128],
                                rhs=h_prev[:, k, :],
                                start=False,
                                stop=(k == 1),
                                skip_group_check=True,
                            )

                # Per-bank sigmoid covers that half's 4 gate blocks
                # (g rows doubled host-side: block 0 holds sigma(2g)).
                for x in (0, 1):
                    if bias_zero:
                        nc.scalar.activation(sg[:, x, :, :], banks[x][:, 0:4, :], sig)
                    else:
                        for b in range(4):
                            nc.scalar.activation(
                                sg[:, x, b, :], banks[x][:, b, :], sig,
                                bias=bias_sb[:, BLK_GATE[b] + x:BLK_GATE[b] + x + 1],
                            )
                for x in (0, 1):
                    # tanh(g) = 2*sigma(2g) - 1 on gpsimd (keeps DVE queue short)
                    nc.gpsimd.tensor_scalar(
                        ctg_cur[:, x, 1, :], sg[:, x, 0, :], 2.0, -1.0,
                        mybir.AluOpType.mult, mybir.AluOpType.add,
                    )
                    # prod = [sf*c_prev, si*tanh_g]
                    nc.vector.tensor_mul(
                        prod[:, x, :, :], sg[:, x, 1:3, :], ctg_cur[:, x, :, :]
                    )
                    nc.vector.tensor_add(
                        ctg_next[:, x, 0, :], prod[:, x, 0, :], prod[:, x, 1, :]
                    )
                # tanh(c) and h split per half: h0 lands early, feeds k0 burst
                nc.scalar.activation(tcn[:, 0, :], ctg_next[:, 0, 0, :], tnh)
                nc.gpsimd.tensor_mul(hn[:, 0, :], sg[:, 0, 3, :], tcn[:, 0, :])
                nc.scalar.activation(tcn[:, 1, :], ctg_next[:, 1, 0, :], tnh)
                nc.vector.tensor_mul(hn[:, 1, :], sg[:, 1, 3, :], tcn[:, 1, :])

                nc.gpsimd.tensor_add(pooled, pooled, hn)
                h_prev = hn
                ctg_cur = ctg_next

            # ---- FC ------------------------------------------------------
            # pooled columns are (row, window)-ordered; FC consumes
            # window-major slices via a strided AP (one-shot, cost ok).
            pooled4 = pooled[:].rearrange("p k (r v) -> p k r v", v=NUM_WIN)
            fps = fc_ps.tile([CLS, BC], F32, tag="fc")
            for idx, (cw, k) in enumerate([(0, 0), (0, 1), (1, 0), (1, 1)]):
                nc.tensor.matmul(
                    out=fps,
                    lhsT=wfcT[:, idx * CLS:(idx + 1) * CLS],
                    rhs=pooled4[:, k, :, cw],
                    start=(idx == 0),
                    stop=(idx == 3),
                )
            out_sb = persist.tile([CLS, BC], F32)
            nc.scalar.copy(out=out_sb, in_=fps)
            nc.sync.dma_start(out=out_d[:], in_=out_sb)

    nc.finalize()
    return nc


_CACHE = {}


def _get_nc(bias_zero: bool):
    if bias_zero not in _CACHE:
        _CACHE[bias_zero] = build(bias_zero)
    return _CACHE[bias_zero]


def host_weights(W_ih, W_hh, W_fc, bias):
    """Host-side weight prep: fold tanh(g)=2*sigmoid(2g)-1 (double g rows),
    transpose into the kernel's tile layouts, cast matmul operands fp16."""
    W_ih = np.asarray(W_ih, np.float32).copy()
    W_hh = np.asarray(W_hh, np.float32).copy()
    bias = np.asarray(bias, np.float32).copy()
    W_ih[2 * H:3 * H] *= 2.0
    W_hh[2 * H:3 * H] *= 2.0
    bias[2 * H:3 * H] *= 2.0
    wihT = np.ascontiguousarray(W_ih.T).astype(np.float16)  # [128, 8*128]
    # whhT[p, (g*2+k)*128 + m] = W_hh[g*128 + m, k*128 + p]
    w = W_hh.reshape(8, 128, 2, 128)  # [g, m, k, p]
    whhT = np.ascontiguousarray(
        w.transpose(3, 0, 2, 1).reshape(128, 16 * 128)
    ).astype(np.float16)
    # wfcT[p, idx*CLS + cls] = W_fc[cls, idx*128 + p]
    wfcT = np.ascontiguousarray(
        np.asarray(W_fc, np.float32).T.reshape(4, 128, CLS)
        .transpose(1, 0, 2).reshape(128, 4 * CLS)
    )
    return wihT, whhT, wfcT, bias


def kernel(x, W_ih, W_hh, b_ih, b_hh, W_fc, b_fc):
    from concourse.bass_utils import run_bass_kernel_spmd

    x = np.asarray(x, dtype=np.float32)
    b_fc = np.asarray(b_fc, dtype=np.float32)
    bias = np.asarray(b_ih, np.float32) + np.asarray(b_hh, np.float32)
    bias_zero = bool(np.all(bias == 0.0))
    nc = _get_nc(bias_zero)

    wihT, whhT, wfcT, bias = host_weights(W_ih, W_hh, W_fc, bias)
    x16 = x.astype(np.float16)

    in_maps = []
    for c in range(NCORES):
        xc = np.ascontiguousarray(x16[c * BC:(c + 1) * BC].reshape(BC * T, C))
        in_maps.append(
            {"x": xc, "wihT": wihT, "whhT": whhT, "wfcT": wfcT, "bias": bias}
        )

    res = run_bass_kernel_spmd(nc, in_maps, list(range(NCORES)))
    out = np.concatenate([r["out"].T for r in res.results], axis=0)
    return (out + b_fc[None, :]).astype(np.float32)


# revision 16
# speedup vs baseline: 1.2032x; 1.0326x over previous
"""Trainium2 Bass kernel for nn_Long_LSTM_Top (2-window masked LSTM + sum-pool + FC).

Strategy (hardcoded for B=256, T=300, C=128, H=256, CLS=60, windows at p=0 and
p=145, each 154 long, over the lag-1 difference sequence d[p] = x[p+1]-x[p]):

- Data-parallel over batch across 8 cores (32 batch rows/core).
- Both windows' LSTM chains run fused: feature dim on partitions and
  (row, window) = 64 columns in the free dim; each weight tile is loaded
  once per step for both windows.
- Weights arrive pre-transposed fp16 from the host; x arrives fp16 and is
  transposed to [c, (r t)] by a single XBAR DMA-transpose. Prep is 4 DMAs
  plus four wide lag-difference subs.
- Per step the 8 gate chunks live in one 2-bank psum tile (half x at block
  offset 8x). The 8 input-projection matmuls lead the banks (first per bank
  start=True, rest overwrite via cleared has_written) and hoist out of the
  h-critical path; W_hh partials run k0-first so the k0 burst of the next
  step overlaps the tail of half-1's chain.
- tanh(g) is folded into the matmul (g rows doubled on host): one merged
  sigmoid over both banks covers all 8 gate blocks; tanh(g) = 2*sg-1,
  c update via paired multiply + pair add, all merged across halves.
  tanh(c) and h = so*tanh(c) stay split per half (h0 lands one ACT+mul
  earlier than h1, feeding the k0 burst early).
- Matmul operands fp16, c state fp32, h fp16. Final FC in fp32.
"""

import numpy as np

import concourse.bass as bass
import concourse.mybir as mybir
from concourse import bacc
from concourse.tile import TileContext

F32 = mybir.dt.float32
F16 = mybir.dt.float16

B, T, C, H, CLS = 256, 300, 128, 256, 60
START, STRIDE, WIN = 1, 145, 154
NUM_WIN = 2
L = T - START  # 299
NCORES = 8
BC = B // NCORES  # 32 rows per core
NSTEP = L  # 299 wall steps
NCOL = NUM_WIN * BC  # 64 scan columns, (row, window) order

# Bank block order [g, f, i, o]; PyTorch gate chunk index for (gate, half x):
# i -> 0+x, f -> 2+x, g -> 4+x, o -> 6+x.
BLK_GATE = [4, 2, 0, 6]


def build(bias_zero: bool = True, nstep: int = NSTEP):
    """Build the per-core Bass module. Returns nc."""
    nc = bacc.Bacc("TRN2", target_bir_lowering=False, debug=False)

    x_d = nc.declare_dram_parameter("x", [BC * T, C], F16, isOutput=False)
    # host-pre-transposed weights: wihT col block = gate chunk,
    # whhT col block = g*2+k, wfcT col block = feature chunk.
    wihT_d = nc.declare_dram_parameter("wihT", [128, 8 * 128], F16, isOutput=False)
    whhT_d = nc.declare_dram_parameter("whhT", [128, 16 * 128], F16, isOutput=False)
    wfcT_d = nc.declare_dram_parameter("wfcT", [128, 4 * CLS], F32, isOutput=False)
    bias_d = nc.declare_dram_parameter("bias", [4 * H], F32, isOutput=False)
    out_d = nc.declare_dram_parameter("out", [CLS, BC], F32, isOutput=True)

    with TileContext(nc) as tc:
        with (
            tc.tile_pool(name="persist", bufs=1) as persist,
            tc.tile_pool(name="ps0", bufs=3, space="PSUM") as ps0_pool,
            tc.tile_pool(name="ps1", bufs=3, space="PSUM") as ps1_pool,
            tc.tile_pool(name="fc_ps", bufs=1, space="PSUM") as fc_ps,
            tc.tile_pool(name="sigp", bufs=3) as sigp,
            tc.tile_pool(name="ctgp", bufs=3) as ctgp,
            tc.tile_pool(name="prodp", bufs=3) as prodp,
            tc.tile_pool(name="tcp", bufs=3) as tcp,
            tc.tile_pool(name="hp", bufs=3) as hp,
        ):
            whhT = persist.tile([128, 16 * 128], F16)
            nc.sync.dma_start(out=whhT, in_=whhT_d[:])
            wihT = persist.tile([128, 8 * 128], F16)
            nc.sync.dma_start(out=wihT, in_=wihT_d[:])
            wfcT = persist.tile([128, 4 * CLS], F32)
            nc.sync.dma_start(out=wfcT, in_=wfcT_d[:])

            # x: single XBAR DMA-transpose [BC*T, C] -> [c, (r t)]
            xT = persist.tile([128, BC * T], F16)
            nc.sync.dma_start_transpose(xT, x_d[:])

            bias_sb = None
            if not bias_zero:
                bias_sb = persist.tile([128, 8], F32)
                nc.sync.dma_start(
                    out=bias_sb, in_=bias_d[:].rearrange("(g p) -> p g", p=128)
                )

            # ---- masked lag-difference, fp16, layout [c, r, win, t] ------
            dm = persist.tile([128, BC, NUM_WIN, NSTEP], F16)
            nc.gpsimd.memset(dm, 0.0)
            xT3 = xT[:].rearrange("p (r t) -> p r t", r=BC)
            # window 0 active at p in [0, 154); window 1 at p in [145, 299).
            for (wwin, lo, hi) in [(0, 0, 80), (1, STRIDE, 224), (0, 80, WIN),
                                   (1, 224, L)]:
                nc.vector.tensor_sub(
                    dm[:, :, wwin, lo:hi],
                    xT3[:, :, lo + 1:hi + 1],
                    xT3[:, :, lo:hi],
                )

            # ---- scan ----------------------------------------------------
            pooled = persist.tile([128, 2, NCOL], F32)
            nc.gpsimd.memset(pooled, 0.0)
            h_prev = hp.tile([128, 2, NCOL], F16, tag="h")
            nc.gpsimd.memset(h_prev, 0.0)
            # ctg tile: per half x, [c_prev, tanh_g] pair blocks.
            ctg_cur = ctgp.tile([128, 2, 2, NCOL], F32, tag="ctg")
            nc.gpsimd.memset(ctg_cur[:, :, 0, :], 0.0)

            sig = mybir.ActivationFunctionType.Sigmoid
            tnh = mybir.ActivationFunctionType.Tanh

            for w in range(nstep):
                rhs_d = dm[:, :, :, w]
                ctg_next = ctgp.tile([128, 2, 2, NCOL], F32, tag="ctg")
                sg = sigp.tile([128, 2, 4, NCOL], F32, tag="sg")
                prod = prodp.tile([128, 2, 2, NCOL], F32, tag="pr")
                tcn = tcp.tile([128, 2, NCOL], F32, tag="tc")
                hn = hp.tile([128, 2, NCOL], F16, tag="h")
                # Two banks: half x in its own bank, blocks [g,f,i,o].
                ps_t0 = ps0_pool.tile([128, 8, NCOL], F32, tag="ps")
                ps_t1 = ps1_pool.tile([128, 8, NCOL], F32, tag="ps")
                banks = (ps_t0, ps_t1)
                # Input-projection matmuls lead each bank: the first clears
                # the bank (start=True), the rest overwrite fresh regions
                # (has_written clear), so all 8 run before h(t-1) exists.
                for x in (0, 1):
                    for b in range(4):
                        gc = BLK_GATE[b] + x
                        nc.tensor.matmul(
                            out=banks[x][:, b, :],
                            lhsT=wihT[:, gc * 128:(gc + 1) * 128],
                            rhs=rhs_d,
                            start=(b == 0),
                            stop=False,
                            skip_group_check=True,
                        )
                # W_hh partials: bank 0's eight first (k0 then k1) so its
                # sigmoid starts while bank 1's matmuls still run.
                for x in (0, 1):
                    for k in (0, 1):
                        for b in range(4):
                            gc = BLK_GATE[b] + x
                            nc.tensor.matmul(
                                out=banks[x][:, b, :],
                                lhsT=whhT[:, (gc * 2 + k) * 128:(gc * 2 + k + 1) * 128],
                                rhs=h_prev[:, k, :],
                                start=False,
                                stop=(k == 1),
                                skip_group_check=True,
                            )

                # Per-bank sigmoid covers that half's 4 gate blocks
                # (g rows doubled host-side: block 0 holds sigma(2g)).
                for x in (0, 1):
                    if bias_zero:
                        nc.scalar.activation(sg[:, x, :, :], banks[x][:, 0:4, :], sig)
                    else:
                        for b in range(4):
                            nc.scalar.activation(
                                sg[:, x, b, :], banks[x][:, b, :], sig,
                                bias=bias_sb[:, BLK_GATE[b] + x:BLK_GATE[b] + x + 1],
                            )
                for x in (0, 1):
                    # tanh(g) = 2*sigma(2g) - 1
                    nc.vector.tensor_scalar(
                        ctg_cur[:, x, 1, :], sg[:, x, 0, :], 2.0, -1.0,
                        mybir.AluOpType.mult, mybir.AluOpType.add,
                    )
                    # prod = [sf*c_prev, si*tanh_g]
                    nc.vector.tensor_mul(
                        prod[:, x, :, :], sg[:, x, 1:3, :], ctg_cur[:, x, :, :]
                    )
                    nc.vector.tensor_add(
                        ctg_next[:, x, 0, :], prod[:, x, 0, :], prod[:, x, 1, :]
                    )
                # tanh(c) and h split per half: h0 lands early, feeds k0 burst
                nc.scalar.activation(tcn[:, 0, :], ctg_next[:, 0, 0, :], tnh)
                nc.gpsimd.tensor_mul(hn[:, 0, :], sg[:, 0, 3, :], tcn[:, 0, :])
                nc.scalar.activation(tcn[:, 1, :], ctg_next[:, 1, 0, :], tnh)
                nc.vector.tensor_mul(hn[:, 1, :], sg[:, 1, 3, :], tcn[:, 1, :])

                nc.gpsimd.tensor_add(pooled, pooled, hn)
                h_prev = hn
                ctg_cur = ctg_next

            # ---- FC ------------------------------------------------------
            # pooled columns are (row, window)-ordered; FC consumes
            # window-major slices via a strided AP (one-shot, cost ok).
            pooled4 = pooled[:].rearrange("p k (r v) -> p k r v", v=NUM_WIN)
            fps = fc_ps.tile([CLS, BC], F32, tag="fc")
            for idx, (cw, k) in enumerate([(0, 0), (0, 1), (1, 0), (1, 1)]):
                nc.tensor.matmul(
                    out=fps,
                    lhsT=wfcT[:, idx * CLS:(idx + 1) * CLS],
                    rhs=pooled4[:, k, :, cw],
                    start=(idx == 0),
                    stop=(idx == 3),
                )
            out_sb = persist.tile([CLS, BC], F32)
            nc.scalar.copy(out=out_sb, in_=fps)
            nc.sync.dma_start(out=out_d[:], in_=out_sb)

    nc.finalize()
    return nc


_CACHE = {}


def _get_nc(bias_zero: bool):
    if bias_zero not in _CACHE:
        _CACHE[bias_zero] = build(bias_zero)
    return _CACHE[bias_zero]


def host_weights(W_ih, W_hh, W_fc, bias):
    """Host-side weight prep: fold tanh(g)=2*sigmoid(2g)-1 (double g rows),
    transpose into the kernel's tile layouts, cast matmul operands fp16."""
    W_ih = np.asarray(W_ih, np.float32).copy()
    W_hh = np.asarray(W_hh, np.float32).copy()
    bias = np.asarray(bias, np.float32).copy()
    W_ih[2 * H:3 * H] *= 2.0
    W_hh[2 * H:3 * H] *= 2.0
    bias[2 * H:3 * H] *= 2.0
    wihT = np.ascontiguousarray(W_ih.T).astype(np.float16)  # [128, 8*128]
    # whhT[p, (g*2+k)*128 + m] = W_hh[g*128 + m, k*128 + p]
    w = W_hh.reshape(8, 128, 2, 128)  # [g, m, k, p]
    whhT = np.ascontiguousarray(
        w.transpose(3, 0, 2, 1).reshape(128, 16 * 128)
    ).astype(np.float16)
    # wfcT[p, idx*CLS + cls] = W_fc[cls, idx*128 + p]
    wfcT = np.ascontiguousarray(
        np.asarray(W_fc, np.float32).T.reshape(4, 128, CLS)
        .transpose(1, 0, 2).reshape(128, 4 * CLS)
    )
    return wihT, whhT, wfcT, bias


def kernel(x, W_ih, W_hh, b_ih, b_hh, W_fc, b_fc):
    from concourse.bass_utils import run_bass_kernel_spmd

    x = np.asarray(x, dtype=np.float32)
    b_fc = np.asarray(b_fc, dtype=np.float32)
    bias = np.asarray(b_ih, np.float32) + np.asarray(b_hh, np.float32)
    bias_zero = bool(np.all(bias == 0.0))
    nc = _get_nc(bias_zero)

    wihT, whhT, wfcT, bias = host_weights(W_ih, W_hh, W_fc, bias)
    x16 = x.astype(np.float16)

    in_maps = []
    for c in range(NCORES):
        xc = np.ascontiguousarray(x16[c * BC:(c + 1) * BC].reshape(BC * T, C))
        in_maps.append(
            {"x": xc, "wihT": wihT, "whhT": whhT, "wfcT": wfcT, "bias": bias}
        )

    res = run_bass_kernel_spmd(nc, in_maps, list(range(NCORES)))
    out = np.concatenate([r["out"].T for r in res.results], axis=0)
    return (out + b_fc[None, :]).astype(np.float32)


# revision 19
# speedup vs baseline: 1.2680x; 1.0539x over previous
"""Trainium2 Bass kernel for nn_Long_LSTM_Top (2-window masked LSTM + sum-pool + FC).

Strategy (hardcoded for B=256, T=300, C=128, H=256, CLS=60, windows at p=0 and
p=145, each 154 long, over the lag-1 difference sequence d[p] = x[p+1]-x[p]):

- Data-parallel over batch across 8 cores (32 batch rows/core).
- Both windows' LSTM chains run fused: feature dim on partitions and
  (row, window) = 64 columns in the free dim; each weight tile is loaded
  once per step for both windows.
- Weights arrive pre-transposed fp16 from the host; x arrives fp16 and is
  transposed to [c, (r t)] by a single XBAR DMA-transpose. Prep is 4 DMAs
  plus four wide lag-difference subs.
- Per step the 8 gate chunks live in one 2-bank psum tile (half x at block
  offset 8x). The 8 input-projection matmuls lead the banks (first per bank
  start=True, rest overwrite via cleared has_written) and hoist out of the
  h-critical path; W_hh partials run k0-first so the k0 burst of the next
  step overlaps the tail of half-1's chain.
- tanh(g) is folded into the matmul (g rows doubled on host): one merged
  sigmoid over both banks covers all 8 gate blocks; tanh(g) = 2*sg-1,
  c update via paired multiply + pair add, all merged across halves.
  tanh(c) and h = so*tanh(c) stay split per half (h0 lands one ACT+mul
  earlier than h1, feeding the k0 burst early).
- Matmul operands fp16, c state fp32, h fp16. Final FC in fp32.
"""

import numpy as np

import concourse.bass as bass
import concourse.mybir as mybir
from concourse import bacc
from concourse.tile import TileContext

F32 = mybir.dt.float32
F16 = mybir.dt.float16

B, T, C, H, CLS = 256, 300, 128, 256, 60
START, STRIDE, WIN = 1, 145, 154
NUM_WIN = 2
L = T - START  # 299
NCORES = 8
BC = B // NCORES  # 32 rows per core
NSTEP = L  # 299 wall steps
NCOL = NUM_WIN * BC  # 64 scan columns, (row, window) order

# Bank block order [g, f, i, o]; PyTorch gate chunk index for (gate, half x):
# i -> 0+x, f -> 2+x, g -> 4+x, o -> 6+x.
BLK_GATE = [4, 2, 0, 6]


def build(bias_zero: bool = True, nstep: int = NSTEP):
    """Build the per-core Bass module. Returns nc."""
    nc = bacc.Bacc("TRN2", target_bir_lowering=False, debug=False)

    x_d = nc.declare_dram_parameter("x", [BC * T, C], F16, isOutput=False)
    # host-pre-transposed weights: wihT col block = gate chunk,
    # whhT col block = g*2+k, wfcT col block = feature chunk.
    wihT_d = nc.declare_dram_parameter("wihT", [128, 8 * 128], F16, isOutput=False)
    whhT_d = nc.declare_dram_parameter("whhT", [128, 16 * 128], F16, isOutput=False)
    wfcT_d = nc.declare_dram_parameter("wfcT", [128, 4 * CLS], F32, isOutput=False)
    bias_d = nc.declare_dram_parameter("bias", [4 * H], F32, isOutput=False)
    out_d = nc.declare_dram_parameter("out", [CLS, BC], F32, isOutput=True)

    with TileContext(nc) as tc:
        with (
            tc.tile_pool(name="persist", bufs=1) as persist,
            tc.tile_pool(name="ps0", bufs=3, space="PSUM") as ps0_pool,
            tc.tile_pool(name="ps1", bufs=3, space="PSUM") as ps1_pool,
            tc.tile_pool(name="fc_ps", bufs=1, space="PSUM") as fc_ps,
            tc.tile_pool(name="sigp", bufs=3) as sigp,
            tc.tile_pool(name="ctgp", bufs=3) as ctgp,
            tc.tile_pool(name="prodp", bufs=3) as prodp,
            tc.tile_pool(name="tcp", bufs=3) as tcp,
            tc.tile_pool(name="hp", bufs=3) as hp,
        ):
            whhT = persist.tile([128, 16 * 128], F16)
            nc.sync.dma_start(out=whhT, in_=whhT_d[:])
            wihT = persist.tile([128, 8 * 128], F16)
            nc.sync.dma_start(out=wihT, in_=wihT_d[:])
            wfcT = persist.tile([128, 4 * CLS], F32)
            nc.sync.dma_start(out=wfcT, in_=wfcT_d[:])

            # x: single XBAR DMA-transpose [BC*T, C] -> [c, (r t)]
            xT = persist.tile([128, BC * T], F16)
            nc.sync.dma_start_transpose(xT, x_d[:])

            bias_sb = None
            if not bias_zero:
                bias_sb = persist.tile([128, 8], F32)
                nc.sync.dma_start(
                    out=bias_sb, in_=bias_d[:].rearrange("(g p) -> p g", p=128)
                )

            # ---- masked lag-difference, fp16, layout [c, r, win, t] ------
            dm = persist.tile([128, NUM_WIN, BC, NSTEP], F16)
            nc.gpsimd.memset(dm, 0.0)
            xT3 = xT[:].rearrange("p (r t) -> p r t", r=BC)
            # window 0 active at p in [0, 154); window 1 at p in [145, 299).
            for (wwin, lo, hi) in [(0, 0, 80), (1, STRIDE, 224), (0, 80, WIN),
                                   (1, 224, L)]:
                nc.vector.tensor_sub(
                    dm[:, wwin, :, lo:hi],
                    xT3[:, :, lo + 1:hi + 1],
                    xT3[:, :, lo:hi],
                )

            # ---- scan ----------------------------------------------------
            pooled = persist.tile([128, 2, NCOL], F32)
            nc.gpsimd.memset(pooled, 0.0)
            # Persistent double/triple buffers (not pool tiles) so the w1
            # column block can stay zero across half-width steps.
            h_bufs = [persist.tile([128, 2, NCOL], F16, name=f"hbuf{i}")
                      for i in range(2)]
            for hb in h_bufs:
                nc.gpsimd.memset(hb, 0.0)
            # ctg tile: per half x, [c_prev, tanh_g] pair blocks.
            ctg_bufs = [persist.tile([128, 2, 2, NCOL], F32, name=f"ctgbuf{i}")
                        for i in range(3)]
            for cb in ctg_bufs:
                nc.gpsimd.memset(cb, 0.0)
            h_prev = h_bufs[1]

            sig = mybir.ActivationFunctionType.Sigmoid
            tnh = mybir.ActivationFunctionType.Tanh

            for w in range(nstep):
                rhs_d = dm[:, :, :, w]
                ctg_cur = ctg_bufs[w % 3]
                ctg_next = ctg_bufs[(w + 1) % 3]
                sg = sigp.tile([128, 2, 4, NCOL], F32, tag="sg")
                prod = prodp.tile([128, 2, 2, NCOL], F32, tag="pr")
                tcn = tcp.tile([128, 2, NCOL], F32, tag="tc")
                hn = h_bufs[w % 2]
                # Two banks: half x in its own bank, blocks [g,f,i,o].
                ps_t0 = ps0_pool.tile([128, 8, NCOL], F32, tag="ps")
                ps_t1 = ps1_pool.tile([128, 8, NCOL], F32, tag="ps")
                banks = (ps_t0, ps_t1)
                # Input-projection matmuls lead each bank: the first clears
                # the bank (start=True), the rest overwrite fresh regions
                # (has_written clear), so all 8 run before h(t-1) exists.
                for x in (0, 1):
                    for b in range(4):
                        gc = BLK_GATE[b] + x
                        nc.tensor.matmul(
                            out=banks[x][:, b, :],
                            lhsT=wihT[:, gc * 128:(gc + 1) * 128],
                            rhs=rhs_d,
                            start=(b == 0),
                            stop=False,
                            skip_group_check=True,
                        )
                # W_hh partials: bank 0's eight first (k0 then k1) so its
                # sigmoid starts while bank 1's matmuls still run.
                for x in (0, 1):
                    for k in (0, 1):
                        for b in range(4):
                            gc = BLK_GATE[b] + x
                            nc.tensor.matmul(
                                out=banks[x][:, b, :],
                                lhsT=whhT[:, (gc * 2 + k) * 128:(gc * 2 + k + 1) * 128],
                                rhs=h_prev[:, k, :],
                                start=False,
                                stop=(k == 1),
                                skip_group_check=True,
                            )

                # Steps before STRIDE have window 1 identically zero:
                # run the tail at half width (w0 columns are contiguous).
                CW = BC if w < STRIDE else NCOL
                # Per-bank sigmoid covers that half's 4 gate blocks
                # (g rows doubled host-side: block 0 holds sigma(2g)).
                for x in (0, 1):
                    if bias_zero:
                        nc.scalar.activation(
                            sg[:, x, :, 0:CW], banks[x][:, 0:4, 0:CW], sig
                        )
                    else:
                        for b in range(4):
                            nc.scalar.activation(
                                sg[:, x, b, 0:CW], banks[x][:, b, 0:CW], sig,
                                bias=bias_sb[:, BLK_GATE[b] + x:BLK_GATE[b] + x + 1],
                            )
                for x in (0, 1):
                    # tanh(g) = 2*sigma(2g) - 1
                    nc.vector.tensor_scalar(
                        ctg_cur[:, x, 1, 0:CW], sg[:, x, 0, 0:CW], 2.0, -1.0,
                        mybir.AluOpType.mult, mybir.AluOpType.add,
                    )
                    # prod = [sf*c_prev, si*tanh_g]
                    nc.vector.tensor_mul(
                        prod[:, x, :, 0:CW], sg[:, x, 1:3, 0:CW],
                        ctg_cur[:, x, :, 0:CW],
                    )
                    nc.vector.tensor_add(
                        ctg_next[:, x, 0, 0:CW], prod[:, x, 0, 0:CW],
                        prod[:, x, 1, 0:CW],
                    )
                # tanh(c) and h split per half: h0 lands early, feeds k0 burst
                nc.scalar.activation(tcn[:, 0, 0:CW], ctg_next[:, 0, 0, 0:CW], tnh)
                nc.gpsimd.tensor_mul(
                    hn[:, 0, 0:CW], sg[:, 0, 3, 0:CW], tcn[:, 0, 0:CW]
                )
                nc.scalar.activation(tcn[:, 1, 0:CW], ctg_next[:, 1, 0, 0:CW], tnh)
                nc.vector.tensor_mul(
                    hn[:, 1, 0:CW], sg[:, 1, 3, 0:CW], tcn[:, 1, 0:CW]
                )
                nc.gpsimd.tensor_add(
                    pooled[:, :, 0:CW], pooled[:, :, 0:CW], hn[:, :, 0:CW]
                )
                h_prev = hn

            # ---- FC ------------------------------------------------------
            # pooled columns are (window, row)-ordered: contiguous slices.
            fps = fc_ps.tile([CLS, BC], F32, tag="fc")
            for idx, (cw, k) in enumerate([(0, 0), (0, 1), (1, 0), (1, 1)]):
                nc.tensor.matmul(
                    out=fps,
                    lhsT=wfcT[:, idx * CLS:(idx + 1) * CLS],
                    rhs=pooled[:, k, cw * BC:(cw + 1) * BC],
                    start=(idx == 0),
                    stop=(idx == 3),
                )
            out_sb = persist.tile([CLS, BC], F32)
            nc.scalar.copy(out=out_sb, in_=fps)
            nc.sync.dma_start(out=out_d[:], in_=out_sb)

    nc.finalize()
    return nc


_CACHE = {}


def _get_nc(bias_zero: bool):
    if bias_zero not in _CACHE:
        _CACHE[bias_zero] = build(bias_zero)
    return _CACHE[bias_zero]


def host_weights(W_ih, W_hh, W_fc, bias):
    """Host-side weight prep: fold tanh(g)=2*sigmoid(2g)-1 (double g rows),
    transpose into the kernel's tile layouts, cast matmul operands fp16."""
    W_ih = np.asarray(W_ih, np.float32).copy()
    W_hh = np.asarray(W_hh, np.float32).copy()
    bias = np.asarray(bias, np.float32).copy()
    W_ih[2 * H:3 * H] *= 2.0
    W_hh[2 * H:3 * H] *= 2.0
    bias[2 * H:3 * H] *= 2.0
    wihT = np.ascontiguousarray(W_ih.T).astype(np.float16)  # [128, 8*128]
    # whhT[p, (g*2+k)*128 + m] = W_hh[g*128 + m, k*128 + p]
    w = W_hh.reshape(8, 128, 2, 128)  # [g, m, k, p]
    whhT = np.ascontiguousarray(
        w.transpose(3, 0, 2, 1).reshape(128, 16 * 128)
    ).astype(np.float16)
    # wfcT[p, idx*CLS + cls] = W_fc[cls, idx*128 + p]
    wfcT = np.ascontiguousarray(
        np.asarray(W_fc, np.float32).T.reshape(4, 128, CLS)
        .transpose(1, 0, 2).reshape(128, 4 * CLS)
    )
    return wihT, whhT, wfcT, bias


def kernel(x, W_ih, W_hh, b_ih, b_hh, W_fc, b_fc):
    from concourse.bass_utils import run_bass_kernel_spmd

    x = np.asarray(x, dtype=np.float32)
    b_fc = np.asarray(b_fc, dtype=np.float32)
    bias = np.asarray(b_ih, np.float32) + np.asarray(b_hh, np.float32)
    bias_zero = bool(np.all(bias == 0.0))
    nc = _get_nc(bias_zero)

    wihT, whhT, wfcT, bias = host_weights(W_ih, W_hh, W_fc, bias)
    x16 = x.astype(np.float16)

    in_maps = []
    for c in range(NCORES):
        xc = np.ascontiguousarray(x16[c * BC:(c + 1) * BC].reshape(BC * T, C))
        in_maps.append(
            {"x": xc, "wihT": wihT, "whhT": whhT, "wfcT": wfcT, "bias": bias}
        )

    res = run_bass_kernel_spmd(nc, in_maps, list(range(NCORES)))
    out = np.concatenate([r["out"].T for r in res.results], axis=0)
    return (out + b_fc[None, :]).astype(np.float32)


# revision 21
# speedup vs baseline: 1.3138x; 1.0362x over previous
"""Trainium2 Bass kernel for nn_Long_LSTM_Top (2-window masked LSTM + sum-pool + FC).

Strategy (hardcoded for B=256, T=300, C=128, H=256, CLS=60, windows at p=0 and
p=145, each 154 long, over the lag-1 difference sequence d[p] = x[p+1]-x[p]):

- Data-parallel over batch across 8 cores (32 batch rows/core).
- Both windows' LSTM chains run fused: feature dim on partitions and
  (row, window) = 64 columns in the free dim; each weight tile is loaded
  once per step for both windows.
- Weights arrive pre-transposed fp16 from the host; x arrives fp16 and is
  transposed to [c, (r t)] by a single XBAR DMA-transpose. Prep is 4 DMAs
  plus four wide lag-difference subs.
- Per step the 8 gate chunks live in one 2-bank psum tile (half x at block
  offset 8x). The 8 input-projection matmuls lead the banks (first per bank
  start=True, rest overwrite via cleared has_written) and hoist out of the
  h-critical path; W_hh partials run k0-first so the k0 burst of the next
  step overlaps the tail of half-1's chain.
- tanh(g) is folded into the matmul (g rows doubled on host): one merged
  sigmoid over both banks covers all 8 gate blocks; tanh(g) = 2*sg-1,
  c update via paired multiply + pair add, all merged across halves.
  tanh(c) and h = so*tanh(c) stay split per half (h0 lands one ACT+mul
  earlier than h1, feeding the k0 burst early).
- Matmul operands fp16, c state fp32, h fp16. Final FC in fp32.
"""

import numpy as np

import concourse.bass as bass
import concourse.mybir as mybir
from concourse import bacc
from concourse.tile import TileContext

F32 = mybir.dt.float32
F16 = mybir.dt.float16

B, T, C, H, CLS = 256, 300, 128, 256, 60
START, STRIDE, WIN = 1, 145, 154
NUM_WIN = 2
L = T - START  # 299
NCORES = 8
BC = B // NCORES  # 32 rows per core
NSTEP = L  # 299 wall steps
NCOL = NUM_WIN * BC  # 64 scan columns, (row, window) order

# Bank block order [g, f, i, o]; PyTorch gate chunk index for (gate, half x):
# i -> 0+x, f -> 2+x, g -> 4+x, o -> 6+x.
BLK_GATE = [4, 2, 0, 6]


def build(bias_zero: bool = True, nstep: int = NSTEP):
    """Build the per-core Bass module. Returns nc."""
    nc = bacc.Bacc("TRN2", target_bir_lowering=False, debug=False)

    x_d = nc.declare_dram_parameter("x", [BC * T, C], F16, isOutput=False)
    # host-pre-transposed weights: wihT col block = gate chunk,
    # whhT col block = g*2+k, wfcT col block = feature chunk.
    wihT_d = nc.declare_dram_parameter("wihT", [128, 8 * 128], F16, isOutput=False)
    whhT_d = nc.declare_dram_parameter("whhT", [128, 16 * 128], F16, isOutput=False)
    wfcT_d = nc.declare_dram_parameter("wfcT", [128, 4 * CLS], F32, isOutput=False)
    bias_d = nc.declare_dram_parameter("bias", [4 * H], F32, isOutput=False)
    out_d = nc.declare_dram_parameter("out", [CLS, BC], F32, isOutput=True)

    with TileContext(nc) as tc:
        with (
            tc.tile_pool(name="persist", bufs=1) as persist,
            tc.tile_pool(name="ps0", bufs=3, space="PSUM") as ps0_pool,
            tc.tile_pool(name="ps1", bufs=3, space="PSUM") as ps1_pool,
            tc.tile_pool(name="fc_ps", bufs=1, space="PSUM") as fc_ps,
            tc.tile_pool(name="sigp", bufs=3) as sigp,
            tc.tile_pool(name="ctgp", bufs=3) as ctgp,
            tc.tile_pool(name="prodp", bufs=3) as prodp,
            tc.tile_pool(name="tcp", bufs=3) as tcp,
            tc.tile_pool(name="hp", bufs=3) as hp,
        ):
            whhT = persist.tile([128, 16 * 128], F16)
            nc.sync.dma_start(out=whhT, in_=whhT_d[:])
            wihT = persist.tile([128, 8 * 128], F16)
            nc.sync.dma_start(out=wihT, in_=wihT_d[:])
            wfcT = persist.tile([128, 4 * CLS], F32)
            nc.sync.dma_start(out=wfcT, in_=wfcT_d[:])

            # x: single XBAR DMA-transpose [BC*T, C] -> [c, (r t)]
            xT = persist.tile([128, BC * T], F16)
            nc.sync.dma_start_transpose(xT, x_d[:])

            bias_sb = None
            if not bias_zero:
                bias_sb = persist.tile([128, 8], F32)
                nc.sync.dma_start(
                    out=bias_sb, in_=bias_d[:].rearrange("(g p) -> p g", p=128)
                )

            # ---- masked lag-difference, fp16, layout [c, r, win, t] ------
            dm = persist.tile([128, NUM_WIN, BC, NSTEP], F16)
            nc.gpsimd.memset(dm, 0.0)
            xT3 = xT[:].rearrange("p (r t) -> p r t", r=BC)
            # window 0 active at p in [0, 154); window 1 at p in [145, 299).
            for (wwin, lo, hi) in [(0, 0, 80), (1, STRIDE, 224), (0, 80, WIN),
                                   (1, 224, L)]:
                nc.vector.tensor_sub(
                    dm[:, wwin, :, lo:hi],
                    xT3[:, :, lo + 1:hi + 1],
                    xT3[:, :, lo:hi],
                )

            # ---- scan ----------------------------------------------------
            pooled = persist.tile([128, 2, NCOL], F32)
            nc.gpsimd.memset(pooled, 0.0)
            # Persistent double/triple buffers (not pool tiles) so the w1
            # column block can stay zero across half-width steps.
            h_bufs = [persist.tile([128, 2, NCOL], F16, name=f"hbuf{i}")
                      for i in range(2)]
            for hb in h_bufs:
                nc.gpsimd.memset(hb, 0.0)
            # ctg tile: per half x, [c_prev, tanh_g] pair blocks.
            ctg_bufs = [persist.tile([128, 2, 2, NCOL], F32, name=f"ctgbuf{i}")
                        for i in range(3)]
            for cb in ctg_bufs:
                nc.gpsimd.memset(cb, 0.0)
            h_prev = h_bufs[1]

            sig = mybir.ActivationFunctionType.Sigmoid
            tnh = mybir.ActivationFunctionType.Tanh

            for w in range(nstep):
                ctg_cur = ctg_bufs[w % 3]
                ctg_next = ctg_bufs[(w + 1) % 3]
                sg = sigp.tile([128, 2, 4, NCOL], F32, tag="sg")
                prod = prodp.tile([128, 2, 2, NCOL], F32, tag="pr")
                tcn = tcp.tile([128, 2, NCOL], F32, tag="tc")
                hn = h_bufs[w % 2]
                # Input projection is half-width before STRIDE (only window
                # 0 has input and the k burst shares the same column region,
                # keeping per-matmul pending-zero state uniform).
                xp0, xp1 = (0, BC) if w < STRIDE else (0, NCOL)
                rhs_d = dm[:, :, :, w].rearrange("p v r -> p (v r)")[:, xp0:xp1]
                # W_hh burst width: window 1 state is exactly zero before
                # STRIDE, so k matmuls run at 32 columns there.
                kc = BC if w < STRIDE else NCOL
                # Two banks: half x in its own bank, blocks [g,f,i,o].
                ps_t0 = ps0_pool.tile([128, 8, NCOL], F32, tag="ps")
                ps_t1 = ps1_pool.tile([128, 8, NCOL], F32, tag="ps")
                banks = (ps_t0, ps_t1)
                # Input-projection matmuls lead each bank: the first clears
                # the bank (start=True: whole-bank pending-zero means the k
                # matmuls overwrite-then-accumulate any untouched region),
                # the rest overwrite fresh regions, all before h(t-1) exists.
                for x in (0, 1):
                    for b in range(4):
                        gc = BLK_GATE[b] + x
                        nc.tensor.matmul(
                            out=banks[x][:, b, xp0:xp1],
                            lhsT=wihT[:, gc * 128:(gc + 1) * 128],
                            rhs=rhs_d,
                            start=(b == 0),
                            stop=False,
                            skip_group_check=True,
                        )
                # W_hh partials: bank 0's eight first (k0 then k1) so its
                # sigmoid starts while bank 1's matmuls still run.
                for x in (0, 1):
                    for k in (0, 1):
                        for b in range(4):
                            gc = BLK_GATE[b] + x
                            nc.tensor.matmul(
                                out=banks[x][:, b, 0:kc],
                                lhsT=whhT[:, (gc * 2 + k) * 128:(gc * 2 + k + 1) * 128],
                                rhs=h_prev[:, k, 0:kc],
                                start=False,
                                stop=(k == 1),
                                skip_group_check=True,
                            )

                # Steps before STRIDE have window 1 identically zero:
                # run the tail at half width (w0 columns are contiguous).
                CW = BC if w < STRIDE else NCOL
                # Per-bank sigmoid covers that half's 4 gate blocks
                # (g rows doubled host-side: block 0 holds sigma(2g)).
                for x in (0, 1):
                    if bias_zero:
                        nc.scalar.activation(
                            sg[:, x, :, 0:CW], banks[x][:, 0:4, 0:CW], sig
                        )
                    else:
                        for b in range(4):
                            nc.scalar.activation(
                                sg[:, x, b, 0:CW], banks[x][:, b, 0:CW], sig,
                                bias=bias_sb[:, BLK_GATE[b] + x:BLK_GATE[b] + x + 1],
                            )
                for x in (0, 1):
                    # tanh(g) = 2*sigma(2g) - 1
                    nc.vector.tensor_scalar(
                        ctg_cur[:, x, 1, 0:CW], sg[:, x, 0, 0:CW], 2.0, -1.0,
                        mybir.AluOpType.mult, mybir.AluOpType.add,
                    )
                    # prod = [sf*c_prev, si*tanh_g]
                    nc.vector.tensor_mul(
                        prod[:, x, :, 0:CW], sg[:, x, 1:3, 0:CW],
                        ctg_cur[:, x, :, 0:CW],
                    )
                    nc.vector.tensor_add(
                        ctg_next[:, x, 0, 0:CW], prod[:, x, 0, 0:CW],
                        prod[:, x, 1, 0:CW],
                    )
                # tanh(c) and h split per half: h0 lands early, feeds k0 burst
                nc.scalar.activation(tcn[:, 0, 0:CW], ctg_next[:, 0, 0, 0:CW], tnh)
                nc.gpsimd.tensor_mul(
                    hn[:, 0, 0:CW], sg[:, 0, 3, 0:CW], tcn[:, 0, 0:CW]
                )
                nc.scalar.activation(tcn[:, 1, 0:CW], ctg_next[:, 1, 0, 0:CW], tnh)
                nc.vector.tensor_mul(
                    hn[:, 1, 0:CW], sg[:, 1, 3, 0:CW], tcn[:, 1, 0:CW]
                )
                nc.gpsimd.tensor_add(
                    pooled[:, :, 0:CW], pooled[:, :, 0:CW], hn[:, :, 0:CW]
                )
                h_prev = hn

            # ---- FC ------------------------------------------------------
            # pooled columns are (window, row)-ordered: contiguous slices.
            fps = fc_ps.tile([CLS, BC], F32, tag="fc")
            for idx, (cw, k) in enumerate([(0, 0), (0, 1), (1, 0), (1, 1)]):
                nc.tensor.matmul(
                    out=fps,
                    lhsT=wfcT[:, idx * CLS:(idx + 1) * CLS],
                    rhs=pooled[:, k, cw * BC:(cw + 1) * BC],
                    start=(idx == 0),
                    stop=(idx == 3),
                )
            out_sb = persist.tile([CLS, BC], F32)
            nc.scalar.copy(out=out_sb, in_=fps)
            nc.sync.dma_start(out=out_d[:], in_=out_sb)

    nc.finalize()
    return nc


_CACHE = {}


def _get_nc(bias_zero: bool):
    if bias_zero not in _CACHE:
        _CACHE[bias_zero] = build(bias_zero)
    return _CACHE[bias_zero]


def host_weights(W_ih, W_hh, W_fc, bias):
    """Host-side weight prep: fold tanh(g)=2*sigmoid(2g)-1 (double g rows),
    transpose into the kernel's tile layouts, cast matmul operands fp16."""
    W_ih = np.asarray(W_ih, np.float32).copy()
    W_hh = np.asarray(W_hh, np.float32).copy()
    bias = np.asarray(bias, np.float32).copy()
    W_ih[2 * H:3 * H] *= 2.0
    W_hh[2 * H:3 * H] *= 2.0
    bias[2 * H:3 * H] *= 2.0
    wihT = np.ascontiguousarray(W_ih.T).astype(np.float16)  # [128, 8*128]
    # whhT[p, (g*2+k)*128 + m] = W_hh[g*128 + m, k*128 + p]
    w = W_hh.reshape(8, 128, 2, 128)  # [g, m, k, p]
    whhT = np.ascontiguousarray(
        w.transpose(3, 0, 2, 1).reshape(128, 16 * 128)
    ).astype(np.float16)
    # wfcT[p, idx*CLS + cls] = W_fc[cls, idx*128 + p]
    wfcT = np.ascontiguousarray(
        np.asarray(W_fc, np.float32).T.reshape(4, 128, CLS)
        .transpose(1, 0, 2).reshape(128, 4 * CLS)
    )
    return wihT, whhT, wfcT, bias


def kernel(x, W_ih, W_hh, b_ih, b_hh, W_fc, b_fc):
    from concourse.bass_utils import run_bass_kernel_spmd

    x = np.asarray(x, dtype=np.float32)
    b_fc = np.asarray(b_fc, dtype=np.float32)
    bias = np.asarray(b_ih, np.float32) + np.asarray(b_hh, np.float32)
    bias_zero = bool(np.all(bias == 0.0))
    nc = _get_nc(bias_zero)

    wihT, whhT, wfcT, bias = host_weights(W_ih, W_hh, W_fc, bias)
    x16 = x.astype(np.float16)

    in_maps = []
    for c in range(NCORES):
        xc = np.ascontiguousarray(x16[c * BC:(c + 1) * BC].reshape(BC * T, C))
        in_maps.append(
            {"x": xc, "wihT": wihT, "whhT": whhT, "wfcT": wfcT, "bias": bias}
        )

    res = run_bass_kernel_spmd(nc, in_maps, list(range(NCORES)))
    out = np.concatenate([r["out"].T for r in res.results], axis=0)
    return (out + b_fc[None, :]).astype(np.float32)
